# revision 17
# baseline (speedup 1.0000x reference)
# Multi-head attention block (QKV proj + per-head q/k layernorm + softmax
# attention + output proj) on 8 Trainium2 NeuronCores.
#
# Sharding: data-parallel over (batch, head-half). Core c handles batch
# c//2 and heads [ (c%2)*8, (c%2)*8+8 ) for ALL 2048 tokens. No duplicated
# K/V compute; each core emits a partial output projection (contraction
# over its 512 features) and the host sums the two partials per batch and
# adds the (bproj + bqkv_v @ Wproj) bias row once.
#
# Per-core dataflow:
#   xT (host-transposed, bf16) stays resident; x-stationary matmuls give
#   k,v,q in token-major [128 tok, 512 feat] PSUM tiles (qkv bias via K=1
#   ones matmul; v bias folded into the host-side bias row).
#   LayerNorm trick: (q-muq).(k-muk) == qhat.(k-muk) when k is centered, and
#   centering of k is folded into the score matmul as an augmented
#   contraction row: row 64 of kT is -mu_k (per kv token), row 64 of qT is
#   sum_d qhat = -(-8 mu_q rstd_q). rstd_k is applied per kv-token via the
#   exp()'s per-partition scale AP, rstd_q*0.125 via one per-partition
#   multiply on q. So k needs NO elementwise normalization at all.
#   q/k head blocks [128 tok, 65] are PE-transposed (bf16 PSUM) and evicted
#   by GPSIMD into feature-major qT/kT [65, head, 2048].
#   Attention per (head, q-half): 16 kv tiles, score matmul (K=65, N=1024),
#   exp on ACT (scale = rstd_k per partition), attn@v with a ones column so
#   Z rides along as PSUM row 64. 1/Z via DVE reciprocal, broadcast to 64
#   partitions with gpsimd.partition_broadcast, one multiply -> yT bf16.
#   Output proj: yT-stationary, Wproj-moving, PSUM DMA'd straight to DRAM.
import contextlib

import numpy as np
import ml_dtypes

B, T, E = 4, 2048, 1024
H, D = 16, 64
P = 128
EPS = 1e-5
SCALE = 0.125         # 1/sqrt(D)
NCORES = 8
HPC = 8               # heads per core
FQ = HPC * D          # 512 features per core (per q/k/v)
KB = E // P           # 8 contraction blocks
MT = T // P           # 16 token tiles
DA = D + 1            # augmented contraction depth (65)
XCH = 8               # x chunks of 256 tokens

_BUILT = {}
_last_in_maps = None


def _build_real():
    import concourse.bass as bass
    import concourse.bacc as bacc
    import concourse.tile as tile
    from concourse import mybir
    from concourse.masks import make_identity

    f32 = mybir.dt.float32
    bf16 = mybir.dt.bfloat16
    AF = mybir.ActivationFunctionType
    OP = mybir.AluOpType

    nc = bacc.Bacc("TRN2", target_bir_lowering=False)
    xT = nc.declare_dram_parameter("xT", [E, T], bf16, isOutput=False)
    wq = nc.declare_dram_parameter("wq", [E, FQ], bf16, isOutput=False)
    wk = nc.declare_dram_parameter("wk", [E, FQ], bf16, isOutput=False)
    wv = nc.declare_dram_parameter("wv", [E, FQ], bf16, isOutput=False)
    wp = nc.declare_dram_parameter("wp", [FQ, E], bf16, isOutput=False)
    bq = nc.declare_dram_parameter("bq", [FQ], bf16, isOutput=False)
    bk = nc.declare_dram_parameter("bk", [FQ], bf16, isOutput=False)
    out = nc.declare_dram_parameter("out", [T, E], f32, isOutput=True)

    with tile.TileContext(nc) as tc, contextlib.ExitStack() as top:
        const = top.enter_context(tc.tile_pool(name="const", bufs=1))
        wpool = top.enter_context(tc.tile_pool(name="wpool", bufs=1))
        big = top.enter_context(tc.tile_pool(name="big", bufs=1))

        ones = const.tile([P, P], bf16)
        nc.vector.memset(ones[:], 1.0)
        ident = const.tile([P, P], bf16)
        make_identity(nc, ident[:])
        eps_t = const.tile([P, 1], f32)
        nc.vector.memset(eps_t[:], EPS)
        # resident weights; wk first (first k-tile needs it), wp last
        wk_sb = wpool.tile([P, KB, FQ], bf16)
        nc.sync.dma_start(out=wk_sb[:], in_=wk[:].rearrange("(kb p) f -> p kb f", p=P))
        bk_row = const.tile([P, FQ], bf16)
        nc.sync.dma_start(out=bk_row[0:1, :], in_=bk[:])
        wv_sb = wpool.tile([P, KB, FQ], bf16)
        nc.sync.dma_start(out=wv_sb[:], in_=wv[:].rearrange("(kb p) f -> p kb f", p=P))
        bq_row = const.tile([P, FQ], bf16)
        nc.sync.dma_start(out=bq_row[0:1, :], in_=bq[:])
        wq_sb = wpool.tile([P, KB, FQ], bf16)
        nc.sync.dma_start(out=wq_sb[:], in_=wq[:].rearrange("(kb p) f -> p kb f", p=P))
        wp_sb = wpool.tile([P, FQ // P, E], bf16)

        # big resident tensors
        qT = big.tile([P, HPC, T], bf16)      # rows 0..64 valid (aug row 64)
        kT = big.tile([P, HPC, T], bf16)
        va = big.tile([P, MT, HPC, DA], bf16)  # v + ones column
        nc.vector.memset(va[:, :, :, DA - 1], 1.0)
        yT = big.tile([P, FQ // P, T], bf16)

        def bcast_free(t, nfree):
            # [P, HPC] -> [P, HPC(stride1), nfree(stride0)] broadcast AP
            a = t[:]
            return bass.AP(tensor=a.tensor, offset=a.offset,
                           ap=[a.ap[0], [1, HPC], [0, nfree]])

        # ---- phase A: QKV + LN prep + transposes ----
        with contextlib.ExitStack() as pa:
            xs = pa.enter_context(tc.tile_pool(name="xs", bufs=1))
            work = pa.enter_context(tc.tile_pool(name="work", bufs=1))
            ps = pa.enter_context(tc.tile_pool(name="psA", bufs=1, space="PSUM"))

            x_sb = []
            for c in range(XCH):
                xc = xs.tile([P, KB, T // XCH], bf16, name=f"x_{c}")
                nc.gpsimd.dma_start(
                    out=xc[:],
                    in_=xT[:, c * (T // XCH):(c + 1) * (T // XCH)].rearrange(
                        "(kb p) t -> p kb t", p=P))
                x_sb.append(xc)

            nc.sync.dma_start(out=wp_sb[:],
                              in_=wp[:].rearrange("(kb p) f -> p kb f", p=P))

            def xtile(m):
                c, r = divmod(m, MT // XCH)
                return x_sb[c][:, :, r * P:(r + 1) * P]

            def proj_psum(m, w_sb, name, bias_row):
                pt = ps.tile([P, FQ], f32, name=name, tag="pqkv", bufs=4)
                xm = xtile(m)
                for kb in range(KB):
                    nc.tensor.matmul(pt[:], xm[:, kb, :], w_sb[:, kb, :],
                                     start=(kb == 0),
                                     stop=(bias_row is None and kb == KB - 1))
                if bias_row is not None:
                    nc.tensor.matmul(pt[:], ones[0:1, :], bias_row[0:1, :],
                                     start=False, stop=True)
                return pt

            def stats(pt, tag):
                st = work.tile([P, HPC, 6], f32, tag=f"st{tag}", bufs=2)
                mv = work.tile([P, HPC, 2], f32, tag=f"mv{tag}", bufs=2)
                for h in range(HPC):
                    nc.vector.bn_stats(out=st[:, h, :], in_=pt[:, h * D:(h + 1) * D])
                for h in range(HPC):
                    nc.vector.bn_aggr(out=mv[:, h, :], in_=st[:, h, :])
                std = work.tile([P, HPC], f32, tag=f"sd{tag}", bufs=2)
                nc.scalar.activation(out=std[:], in_=mv[:, :, 1],
                                     func=AF.Sqrt, bias=eps_t[:])
                rstd = work.tile([P, HPC], f32, tag=f"rs{tag}", bufs=2)
                nc.vector.reciprocal_approx_fast(out=rstd[:], in_=std[:])
                return mv, rstd

            def transp_out(src, dstT, m):
                # src [P, HPC, DA] bf16 -> per-head PE transpose -> dstT
                tp = ps.tile([P, HPC, P], bf16, name=f"tp_{dstT}_{m}",
                             tag="tp", bufs=2)
                for h in range(HPC):
                    nc.tensor.transpose(tp[0:DA, h, :], src[:, h, :], ident[:])
                nc.scalar.copy(out=dstT[0:DA, :, m * P:(m + 1) * P],
                               in_=tp[0:DA, :, :])

            for m in range(MT):
                # k
                kp = proj_psum(m, wk_sb, f"kp_{m}", bk_row)
                mvk, rstdk = stats(kp, "k")
                rkb = work.tile([P, HPC], bf16, tag="rkb", bufs=2)
                nc.vector.tensor_copy(out=rkb[:], in_=rstdk[:])
                kraw = work.tile([P, HPC, D], bf16, tag="kraw", bufs=2)
                nc.scalar.copy(
                    out=kraw[:],
                    in_=kp[:].rearrange("p (h d) -> p h d", h=HPC))
                kx = work.tile([P, HPC, DA], bf16, tag="kx", bufs=2)
                nc.gpsimd.tensor_tensor(
                    out=kx[:, :, 0:D], in0=kraw[:],
                    in1=bcast_free(rkb, D), op=OP.mult)
                t2 = work.tile([P, HPC], f32, tag="t2", bufs=2)
                nc.vector.tensor_tensor(out=t2[:], in0=mvk[:, :, 0],
                                        in1=rstdk[:], op=OP.mult)
                nc.vector.tensor_scalar(out=kx[:, :, D], in0=t2[:],
                                        scalar1=-1.0, scalar2=None, op0=OP.mult)
                transp_out(kx, kT, m)
                # v
                vp = proj_psum(m, wv_sb, f"vp_{m}", None)
                nc.scalar.copy(
                    out=va[:, m, :, 0:D],
                    in_=vp[:].rearrange("p (h d) -> p h d", h=HPC))

                # q (interleaved with k/v for chain overlap)
                qp = proj_psum(m, wq_sb, f"qp_{m}", bq_row)
                mvq, rstdq = stats(qp, "q")
                rq = work.tile([P, HPC], bf16, tag="rq", bufs=2)
                nc.vector.tensor_scalar(out=rq[:], in0=rstdq[:],
                                        scalar1=SCALE, scalar2=None, op0=OP.mult)
                qraw = work.tile([P, HPC, D], bf16, tag="qraw", bufs=2)
                nc.scalar.copy(out=qraw[:],
                               in_=qp[:].rearrange("p (h d) -> p h d", h=HPC))
                qx = work.tile([P, HPC, DA], bf16, tag="qx", bufs=2)
                nc.gpsimd.tensor_tensor(
                    out=qx[:, :, 0:D], in0=qraw[:],
                    in1=bcast_free(rq, D), op=OP.mult)
                t1 = work.tile([P, HPC], f32, tag="t1", bufs=2)
                nc.vector.tensor_tensor(out=t1[:], in0=mvq[:, :, 0],
                                        in1=rstdq[:], op=OP.mult)
                nc.vector.tensor_scalar(out=qx[:, :, D], in0=t1[:],
                                        scalar1=float(D) * SCALE, scalar2=None,
                                        op0=OP.mult)
                transp_out(qx, qT, m)

        # ---- phase C: attention ----
        NQ = 1024  # query chunk (PSUM: scr 2x2 banks + py 2x2 banks)
        with contextlib.ExitStack() as pc:
            cw = pc.enter_context(tc.tile_pool(name="cw", bufs=1))
            ps = pc.enter_context(tc.tile_pool(name="psC", bufs=1, space="PSUM"))
            for h in range(HPC):
                r0 = (h % 2) * D
                fb = h // 2
                for qc in range(T // NQ):
                    qsl = slice(qc * NQ, (qc + 1) * NQ)
                    py = ps.tile([P, NQ], f32, name=f"py_{h}_{qc}", tag="py", bufs=2)
                    p_tiles = [None] * MT

                    def emit_score(st):
                        s = ps.tile([P, NQ], f32, name=f"s_{h}_{qc}_{st}",
                                    tag="scr", bufs=2)
                        for nk in range(NQ // 512):
                            nsl = slice(nk * 512, (nk + 1) * 512)
                            nc.tensor.matmul(
                                s[:, nsl], kT[0:DA, h, st * P:(st + 1) * P],
                                qT[0:DA, h, qc * NQ + nk * 512:
                                   qc * NQ + (nk + 1) * 512],
                                start=True, stop=True)
                        p = cw.tile([P, NQ], bf16, tag="p_bf", bufs=6)
                        nc.scalar.activation(out=p[:], in_=s[:], func=AF.Exp)
                        p_tiles[st] = p

                    def emit_av(st):
                        for nk in range(NQ // 512):
                            nsl = slice(nk * 512, (nk + 1) * 512)
                            nc.tensor.matmul(py[0:DA, nsl], va[:, st, h, :],
                                             p_tiles[st][:, nsl],
                                             start=(st == 0), stop=(st == MT - 1))

                    emit_score(0)
                    for st in range(1, MT):
                        emit_score(st)
                        emit_av(st - 1)
                    emit_av(MT - 1)
                    zrec = cw.tile([P, NQ], f32, tag="zrec", bufs=2)
                    nc.vector.reciprocal(out=zrec[0:1, :], in_=py[D:D + 1, :])
                    zrep = cw.tile([D, NQ], f32, tag="zrep", bufs=2)
                    nc.gpsimd.partition_broadcast(zrep[:], zrec[0:1, :])
                    nc.vector.tensor_tensor(out=yT[r0:r0 + D, fb, qsl],
                                            in0=py[0:D, :], in1=zrep[:],
                                            op=OP.mult)

        # ---- phase D: output projection (partial sums, f32 to DRAM) ----
        with contextlib.ExitStack() as pd:
            dw = pd.enter_context(tc.tile_pool(name="dw", bufs=1))
            ps = pd.enter_context(tc.tile_pool(name="psD", bufs=1, space="PSUM"))
            for m in range(MT):
                po = ps.tile([P, E], f32, name=f"po_{m}", tag="po", bufs=4)
                for kb in range(FQ // P):
                    for nk in range(E // 512):
                        nsl = slice(nk * 512, (nk + 1) * 512)
                        nc.tensor.matmul(po[:, nsl],
                                         yT[:, kb, m * P:(m + 1) * P],
                                         wp_sb[:, kb, nsl],
                                         start=(kb == 0), stop=(kb == FQ // P - 1))
                osb = dw.tile([P, E], f32, tag="osb", bufs=4)
                if m % 2 == 0:
                    nc.scalar.copy(out=osb[:], in_=po[:])
                else:
                    nc.vector.tensor_copy(out=osb[:], in_=po[:])
                eng = nc.sync if m % 2 == 0 else nc.gpsimd
                eng.dma_start(out=out[m * P:(m + 1) * P, :], in_=osb[:])

    nc.finalize()
    return nc


def _get_nc():
    if False not in _BUILT:
        _BUILT[False] = _build_real()
    return _BUILT[False]


def _np_fallback(x, Wqkv, bqkv, q_gamma, q_beta, k_gamma, k_beta, Wproj, bproj):
    Bb, Tt, Ee = x.shape
    Dd = Ee // H
    qkv = x.astype(np.float32) @ Wqkv + bqkv
    q, k, v = np.split(qkv, 3, axis=-1)
    th = lambda a: a.reshape(Bb, Tt, H, Dd).transpose(0, 2, 1, 3)
    q, k, v = th(q), th(k), th(v)

    def ln(a, g, b):
        mu = a.mean(-1, keepdims=True)
        var = a.var(-1, keepdims=True)
        return (a - mu) / np.sqrt(var + EPS) * g + b

    q = ln(q, q_gamma, q_beta)
    k = ln(k, k_gamma, k_beta)
    s = np.einsum("bhtd,bhsd->bhts", q, k) / np.sqrt(Dd)
    s -= s.max(-1, keepdims=True)
    p = np.exp(s)
    p /= p.sum(-1, keepdims=True)
    y = np.einsum("bhts,bhsd->bhtd", p, v)
    y = y.transpose(0, 2, 1, 3).reshape(Bb, Tt, Ee)
    return (y @ Wproj + bproj).astype(np.float32)


def kernel(x, Wqkv, bqkv, q_gamma, q_beta, k_gamma, k_beta, Wproj, bproj):
    from concourse.bass_utils import run_bass_kernel_spmd

    x = np.asarray(x, dtype=np.float32)
    Wqkv = np.asarray(Wqkv, dtype=np.float32)
    bqkv = np.asarray(bqkv, dtype=np.float32)
    Wproj = np.asarray(Wproj, dtype=np.float32)
    bproj = np.asarray(bproj, dtype=np.float32)
    q_gamma = np.asarray(q_gamma, dtype=np.float32)
    q_beta = np.asarray(q_beta, dtype=np.float32)
    k_gamma = np.asarray(k_gamma, dtype=np.float32)
    k_beta = np.asarray(k_beta, dtype=np.float32)

    affine = not (np.all(q_gamma == 1.0) and np.all(q_beta == 0.0)
                  and np.all(k_gamma == 1.0) and np.all(k_beta == 0.0))
    if affine:
        return _np_fallback(x, Wqkv, bqkv, q_gamma, q_beta, k_gamma, k_beta,
                            Wproj, bproj)

    nc = _get_nc()
    bf = ml_dtypes.bfloat16

    in_maps = []
    shared = {}
    for hh in range(2):
        fsl = slice(hh * FQ, (hh + 1) * FQ)
        shared[hh] = {
            "wq": np.ascontiguousarray(Wqkv[:, fsl].astype(bf)),
            "wk": np.ascontiguousarray(Wqkv[:, E + hh * FQ:E + (hh + 1) * FQ].astype(bf)),
            "wv": np.ascontiguousarray(Wqkv[:, 2 * E + hh * FQ:2 * E + (hh + 1) * FQ].astype(bf)),
            "wp": np.ascontiguousarray(Wproj[fsl, :].astype(bf)),
            "bq": np.ascontiguousarray(bqkv[fsl].astype(bf)),
            "bk": np.ascontiguousarray(bqkv[E + hh * FQ:E + (hh + 1) * FQ].astype(bf)),
        }
    xTb = [np.ascontiguousarray(x[b].T.astype(bf)) for b in range(B)]
    for c in range(NCORES):
        b, hh = divmod(c, 2)
        m = {"xT": xTb[b]}
        m.update(shared[hh])
        in_maps.append(m)

    global _last_in_maps
    _last_in_maps = in_maps
    res = run_bass_kernel_spmd(nc, in_maps, core_ids=list(range(NCORES)))

    bias_row = (bqkv[2 * E:] @ Wproj + bproj).astype(np.float32)
    y = np.empty((B, T, E), dtype=np.float32)
    for b in range(B):
        y[b] = res.results[2 * b]["out"]
        y[b] += res.results[2 * b + 1]["out"]
        y[b] += bias_row
    return y


# revision 18
# speedup vs baseline: 1.0164x; 1.0164x over previous
# Multi-head attention block (QKV proj + per-head q/k layernorm + softmax
# attention + output proj) on 8 Trainium2 NeuronCores.
#
# Sharding: data-parallel over (batch, head-half). Core c handles batch
# c//2 and heads [ (c%2)*8, (c%2)*8+8 ) for ALL 2048 tokens. No duplicated
# K/V compute; each core emits a partial output projection (contraction
# over its 512 features) and the host sums the two partials per batch and
# adds the (bproj + bqkv_v @ Wproj) bias row once.
#
# Per-core dataflow:
#   xT (host-transposed, bf16) stays resident; x-stationary matmuls give
#   k,v,q in token-major [128 tok, 512 feat] PSUM tiles (qkv bias via K=1
#   ones matmul; v bias folded into the host-side bias row).
#   LayerNorm trick: (q-muq).(k-muk) == qhat.(k-muk) when k is centered, and
#   centering of k is folded into the score matmul as an augmented
#   contraction row: row 64 of kT is -mu_k (per kv token), row 64 of qT is
#   sum_d qhat = -(-8 mu_q rstd_q). rstd_k is applied per kv-token via the
#   exp()'s per-partition scale AP, rstd_q*0.125 via one per-partition
#   multiply on q. So k needs NO elementwise normalization at all.
#   q/k head blocks [128 tok, 65] are PE-transposed (bf16 PSUM) and evicted
#   by GPSIMD into feature-major qT/kT [65, head, 2048].
#   Attention per (head, q-half): 16 kv tiles, score matmul (K=65, N=1024),
#   exp on ACT (scale = rstd_k per partition), attn@v with a ones column so
#   Z rides along as PSUM row 64. 1/Z via DVE reciprocal, broadcast to 64
#   partitions with gpsimd.partition_broadcast, one multiply -> yT bf16.
#   Output proj: yT-stationary, Wproj-moving, PSUM DMA'd straight to DRAM.
import contextlib

import numpy as np
import ml_dtypes

B, T, E = 4, 2048, 1024
H, D = 16, 64
P = 128
EPS = 1e-5
SCALE = 0.125         # 1/sqrt(D)
NCORES = 8
HPC = 8               # heads per core
FQ = HPC * D          # 512 features per core (per q/k/v)
KB = E // P           # 8 contraction blocks
MT = T // P           # 16 token tiles
DA = D + 1            # augmented contraction depth (65)
XCH = 8               # x chunks of 256 tokens

_BUILT = {}
_last_in_maps = None


def _build_real():
    import concourse.bass as bass
    import concourse.bacc as bacc
    import concourse.tile as tile
    from concourse import mybir
    from concourse.masks import make_identity

    f32 = mybir.dt.float32
    bf16 = mybir.dt.bfloat16
    AF = mybir.ActivationFunctionType
    OP = mybir.AluOpType

    nc = bacc.Bacc("TRN2", target_bir_lowering=False)
    xT = nc.declare_dram_parameter("xT", [E, T], bf16, isOutput=False)
    wq = nc.declare_dram_parameter("wq", [E, FQ], bf16, isOutput=False)
    wk = nc.declare_dram_parameter("wk", [E, FQ], bf16, isOutput=False)
    wv = nc.declare_dram_parameter("wv", [E, FQ], bf16, isOutput=False)
    wp = nc.declare_dram_parameter("wp", [FQ, E], bf16, isOutput=False)
    bq = nc.declare_dram_parameter("bq", [FQ], bf16, isOutput=False)
    bk = nc.declare_dram_parameter("bk", [FQ], bf16, isOutput=False)
    out = nc.declare_dram_parameter("out", [T, E], f32, isOutput=True)

    with tile.TileContext(nc) as tc, contextlib.ExitStack() as top:
        const = top.enter_context(tc.tile_pool(name="const", bufs=1))
        wpool = top.enter_context(tc.tile_pool(name="wpool", bufs=1))
        big = top.enter_context(tc.tile_pool(name="big", bufs=1))

        ones = const.tile([P, P], bf16)
        nc.vector.memset(ones[:], 1.0)
        ident = const.tile([P, P], bf16)
        make_identity(nc, ident[:])
        eps_t = const.tile([P, 1], f32)
        nc.vector.memset(eps_t[:], EPS)
        # resident weights; wk first (first k-tile needs it), wp last
        wk_sb = wpool.tile([P, KB, FQ], bf16)
        nc.sync.dma_start(out=wk_sb[:], in_=wk[:].rearrange("(kb p) f -> p kb f", p=P))
        bk_row = const.tile([P, FQ], bf16)
        nc.sync.dma_start(out=bk_row[0:1, :], in_=bk[:])
        wv_sb = wpool.tile([P, KB, FQ], bf16)
        nc.sync.dma_start(out=wv_sb[:], in_=wv[:].rearrange("(kb p) f -> p kb f", p=P))
        bq_row = const.tile([P, FQ], bf16)
        nc.sync.dma_start(out=bq_row[0:1, :], in_=bq[:])
        wq_sb = wpool.tile([P, KB, FQ], bf16)
        nc.sync.dma_start(out=wq_sb[:], in_=wq[:].rearrange("(kb p) f -> p kb f", p=P))
        wp_sb = wpool.tile([P, FQ // P, E], bf16)

        # big resident tensors
        qT = big.tile([P, HPC, T], bf16)      # rows 0..64 valid (aug row 64)
        kT = big.tile([P, HPC, T], bf16)
        va = big.tile([P, MT, HPC, DA], bf16)  # v + ones column
        nc.vector.memset(va[:, :, :, DA - 1], 1.0)
        yT = big.tile([P, FQ // P, T], bf16)

        def bcast_free(t, nfree):
            # [P, HPC] -> [P, HPC(stride1), nfree(stride0)] broadcast AP
            a = t[:]
            return bass.AP(tensor=a.tensor, offset=a.offset,
                           ap=[a.ap[0], [1, HPC], [0, nfree]])

        # ---- phase A: QKV + LN prep + transposes ----
        with contextlib.ExitStack() as pa:
            xs = pa.enter_context(tc.tile_pool(name="xs", bufs=1))
            work = pa.enter_context(tc.tile_pool(name="work", bufs=1))
            ps = pa.enter_context(tc.tile_pool(name="psA", bufs=1, space="PSUM"))

            x_sb = []
            for c in range(XCH):
                xc = xs.tile([P, KB, T // XCH], bf16, name=f"x_{c}")
                nc.gpsimd.dma_start(
                    out=xc[:],
                    in_=xT[:, c * (T // XCH):(c + 1) * (T // XCH)].rearrange(
                        "(kb p) t -> p kb t", p=P))
                x_sb.append(xc)

            nc.sync.dma_start(out=wp_sb[:],
                              in_=wp[:].rearrange("(kb p) f -> p kb f", p=P))

            def xtile(m):
                c, r = divmod(m, MT // XCH)
                return x_sb[c][:, :, r * P:(r + 1) * P]

            def proj_psum(m, w_sb, name, bias_row):
                pt = ps.tile([P, FQ], f32, name=name, tag="pqkv", bufs=4)
                xm = xtile(m)
                for kb in range(KB):
                    nc.tensor.matmul(pt[:], xm[:, kb, :], w_sb[:, kb, :],
                                     start=(kb == 0),
                                     stop=(bias_row is None and kb == KB - 1))
                if bias_row is not None:
                    nc.tensor.matmul(pt[:], ones[0:1, :], bias_row[0:1, :],
                                     start=False, stop=True)
                return pt

            def stats(pt, tag):
                st = work.tile([P, HPC, 6], f32, tag=f"st{tag}", bufs=2)
                mv = work.tile([P, HPC, 2], f32, tag=f"mv{tag}", bufs=2)
                for h in range(HPC):
                    nc.vector.bn_stats(out=st[:, h, :], in_=pt[:, h * D:(h + 1) * D])
                for h in range(HPC):
                    nc.vector.bn_aggr(out=mv[:, h, :], in_=st[:, h, :])
                std = work.tile([P, HPC], f32, tag=f"sd{tag}", bufs=2)
                nc.scalar.activation(out=std[:], in_=mv[:, :, 1],
                                     func=AF.Sqrt, bias=eps_t[:])
                rstd = work.tile([P, HPC], f32, tag=f"rs{tag}", bufs=2)
                nc.vector.reciprocal_approx_fast(out=rstd[:], in_=std[:])
                return mv, rstd

            def transp_out(src, dstT, m):
                # src [P, HPC, DA] bf16 -> per-head PE transpose -> dstT
                tp = ps.tile([P, HPC, P], bf16, name=f"tp_{dstT}_{m}",
                             tag="tp", bufs=2)
                for h in range(HPC):
                    nc.tensor.transpose(tp[0:DA, h, :], src[:, h, :], ident[:])
                nc.scalar.copy(out=dstT[0:DA, :, m * P:(m + 1) * P],
                               in_=tp[0:DA, :, :])

            for m in range(MT):
                # k
                kp = proj_psum(m, wk_sb, f"kp_{m}", bk_row)
                mvk, rstdk = stats(kp, "k")
                rkb = work.tile([P, HPC], bf16, tag="rkb", bufs=2)
                nc.vector.tensor_copy(out=rkb[:], in_=rstdk[:])
                kraw = work.tile([P, HPC, D], bf16, tag="kraw", bufs=2)
                nc.scalar.copy(
                    out=kraw[:],
                    in_=kp[:].rearrange("p (h d) -> p h d", h=HPC))
                kx = work.tile([P, HPC, DA], bf16, tag="kx", bufs=2)
                nc.gpsimd.tensor_tensor(
                    out=kx[:, :, 0:D], in0=kraw[:],
                    in1=bcast_free(rkb, D), op=OP.mult)
                t2 = work.tile([P, HPC], f32, tag="t2", bufs=2)
                nc.vector.tensor_tensor(out=t2[:], in0=mvk[:, :, 0],
                                        in1=rstdk[:], op=OP.mult)
                nc.vector.tensor_scalar(out=kx[:, :, D], in0=t2[:],
                                        scalar1=-1.0, scalar2=None, op0=OP.mult)
                transp_out(kx, kT, m)
                # v
                vp = proj_psum(m, wv_sb, f"vp_{m}", None)
                nc.scalar.copy(
                    out=va[:, m, :, 0:D],
                    in_=vp[:].rearrange("p (h d) -> p h d", h=HPC))

            for m in range(MT):
                # q
                qp = proj_psum(m, wq_sb, f"qp_{m}", bq_row)
                mvq, rstdq = stats(qp, "q")
                rq = work.tile([P, HPC], bf16, tag="rq", bufs=2)
                nc.vector.tensor_scalar(out=rq[:], in0=rstdq[:],
                                        scalar1=SCALE, scalar2=None, op0=OP.mult)
                qraw = work.tile([P, HPC, D], bf16, tag="qraw", bufs=2)
                nc.scalar.copy(out=qraw[:],
                               in_=qp[:].rearrange("p (h d) -> p h d", h=HPC))
                qx = work.tile([P, HPC, DA], bf16, tag="qx", bufs=2)
                nc.gpsimd.tensor_tensor(
                    out=qx[:, :, 0:D], in0=qraw[:],
                    in1=bcast_free(rq, D), op=OP.mult)
                t1 = work.tile([P, HPC], f32, tag="t1", bufs=2)
                nc.vector.tensor_tensor(out=t1[:], in0=mvq[:, :, 0],
                                        in1=rstdq[:], op=OP.mult)
                nc.vector.tensor_scalar(out=qx[:, :, D], in0=t1[:],
                                        scalar1=float(D) * SCALE, scalar2=None,
                                        op0=OP.mult)
                transp_out(qx, qT, m)

        # ---- phase C: attention ----
        NQ = 1024  # query chunk (PSUM: scr 2x2 banks + py 2x2 banks)
        with contextlib.ExitStack() as pc:
            cw = pc.enter_context(tc.tile_pool(name="cw", bufs=1))
            ps = pc.enter_context(tc.tile_pool(name="psC", bufs=1, space="PSUM"))
            for h in range(HPC):
                r0 = (h % 2) * D
                fb = h // 2
                for qc in range(T // NQ):
                    qsl = slice(qc * NQ, (qc + 1) * NQ)
                    py = ps.tile([P, NQ], f32, name=f"py_{h}_{qc}", tag="py", bufs=2)
                    p_tiles = [None] * MT

                    def emit_score(st):
                        s = ps.tile([P, NQ], f32, name=f"s_{h}_{qc}_{st}",
                                    tag="scr", bufs=2)
                        for nk in range(NQ // 512):
                            nsl = slice(nk * 512, (nk + 1) * 512)
                            nc.tensor.matmul(
                                s[:, nsl], kT[0:DA, h, st * P:(st + 1) * P],
                                qT[0:DA, h, qc * NQ + nk * 512:
                                   qc * NQ + (nk + 1) * 512],
                                start=True, stop=True)
                        p = cw.tile([P, NQ], bf16, tag="p_bf", bufs=6)
                        nc.scalar.activation(out=p[:], in_=s[:], func=AF.Exp)
                        p_tiles[st] = p

                    def emit_av(st):
                        for nk in range(NQ // 512):
                            nsl = slice(nk * 512, (nk + 1) * 512)
                            nc.tensor.matmul(py[0:DA, nsl], va[:, st, h, :],
                                             p_tiles[st][:, nsl],
                                             start=(st == 0), stop=(st == MT - 1))

                    emit_score(0)
                    for st in range(1, MT):
                        emit_score(st)
                        emit_av(st - 1)
                    emit_av(MT - 1)
                    zrec = cw.tile([P, NQ], f32, tag="zrec", bufs=2)
                    nc.vector.reciprocal(out=zrec[0:1, :], in_=py[D:D + 1, :])
                    zrep = cw.tile([D, NQ], f32, tag="zrep", bufs=2)
                    nc.gpsimd.partition_broadcast(zrep[:], zrec[0:1, :])
                    nc.vector.tensor_tensor(out=yT[r0:r0 + D, fb, qsl],
                                            in0=py[0:D, :], in1=zrep[:],
                                            op=OP.mult)

        # ---- phase D: output projection (partial sums, f32 to DRAM) ----
        with contextlib.ExitStack() as pd:
            dw = pd.enter_context(tc.tile_pool(name="dw", bufs=1))
            ps = pd.enter_context(tc.tile_pool(name="psD", bufs=1, space="PSUM"))
            for m in range(MT):
                po = ps.tile([P, E], f32, name=f"po_{m}", tag="po", bufs=4)
                for kb in range(FQ // P):
                    for nk in range(E // 512):
                        nsl = slice(nk * 512, (nk + 1) * 512)
                        nc.tensor.matmul(po[:, nsl],
                                         yT[:, kb, m * P:(m + 1) * P],
                                         wp_sb[:, kb, nsl],
                                         start=(kb == 0), stop=(kb == FQ // P - 1))
                osb = dw.tile([P, E], f32, tag="osb", bufs=4)
                if m % 2 == 0:
                    nc.scalar.copy(out=osb[:], in_=po[:])
                else:
                    nc.vector.tensor_copy(out=osb[:], in_=po[:])
                eng = nc.sync if m % 2 == 0 else nc.gpsimd
                eng.dma_start(out=out[m * P:(m + 1) * P, :], in_=osb[:])

    nc.finalize()
    return nc


def _get_nc():
    if False not in _BUILT:
        _BUILT[False] = _build_real()
    return _BUILT[False]


def _np_fallback(x, Wqkv, bqkv, q_gamma, q_beta, k_gamma, k_beta, Wproj, bproj):
    Bb, Tt, Ee = x.shape
    Dd = Ee // H
    qkv = x.astype(np.float32) @ Wqkv + bqkv
    q, k, v = np.split(qkv, 3, axis=-1)
    th = lambda a: a.reshape(Bb, Tt, H, Dd).transpose(0, 2, 1, 3)
    q, k, v = th(q), th(k), th(v)

    def ln(a, g, b):
        mu = a.mean(-1, keepdims=True)
        var = a.var(-1, keepdims=True)
        return (a - mu) / np.sqrt(var + EPS) * g + b

    q = ln(q, q_gamma, q_beta)
    k = ln(k, k_gamma, k_beta)
    s = np.einsum("bhtd,bhsd->bhts", q, k) / np.sqrt(Dd)
    s -= s.max(-1, keepdims=True)
    p = np.exp(s)
    p /= p.sum(-1, keepdims=True)
    y = np.einsum("bhts,bhsd->bhtd", p, v)
    y = y.transpose(0, 2, 1, 3).reshape(Bb, Tt, Ee)
    return (y @ Wproj + bproj).astype(np.float32)


def kernel(x, Wqkv, bqkv, q_gamma, q_beta, k_gamma, k_beta, Wproj, bproj):
    from concourse.bass_utils import run_bass_kernel_spmd

    x = np.asarray(x, dtype=np.float32)
    Wqkv = np.asarray(Wqkv, dtype=np.float32)
    bqkv = np.asarray(bqkv, dtype=np.float32)
    Wproj = np.asarray(Wproj, dtype=np.float32)
    bproj = np.asarray(bproj, dtype=np.float32)
    q_gamma = np.asarray(q_gamma, dtype=np.float32)
    q_beta = np.asarray(q_beta, dtype=np.float32)
    k_gamma = np.asarray(k_gamma, dtype=np.float32)
    k_beta = np.asarray(k_beta, dtype=np.float32)

    affine = not (np.all(q_gamma == 1.0) and np.all(q_beta == 0.0)
                  and np.all(k_gamma == 1.0) and np.all(k_beta == 0.0))
    if affine:
        return _np_fallback(x, Wqkv, bqkv, q_gamma, q_beta, k_gamma, k_beta,
                            Wproj, bproj)

    nc = _get_nc()
    bf = ml_dtypes.bfloat16

    in_maps = []
    shared = {}
    for hh in range(2):
        fsl = slice(hh * FQ, (hh + 1) * FQ)
        shared[hh] = {
            "wq": np.ascontiguousarray(Wqkv[:, fsl].astype(bf)),
            "wk": np.ascontiguousarray(Wqkv[:, E + hh * FQ:E + (hh + 1) * FQ].astype(bf)),
            "wv": np.ascontiguousarray(Wqkv[:, 2 * E + hh * FQ:2 * E + (hh + 1) * FQ].astype(bf)),
            "wp": np.ascontiguousarray(Wproj[fsl, :].astype(bf)),
            "bq": np.ascontiguousarray(bqkv[fsl].astype(bf)),
            "bk": np.ascontiguousarray(bqkv[E + hh * FQ:E + (hh + 1) * FQ].astype(bf)),
        }
    xTb = [np.ascontiguousarray(x[b].T.astype(bf)) for b in range(B)]
    for c in range(NCORES):
        b, hh = divmod(c, 2)
        m = {"xT": xTb[b]}
        m.update(shared[hh])
        in_maps.append(m)

    global _last_in_maps
    _last_in_maps = in_maps
    res = run_bass_kernel_spmd(nc, in_maps, core_ids=list(range(NCORES)))

    bias_row = (bqkv[2 * E:] @ Wproj + bproj).astype(np.float32)
    y = np.empty((B, T, E), dtype=np.float32)
    for b in range(B):
        y[b] = res.results[2 * b]["out"]
        y[b] += res.results[2 * b + 1]["out"]
        y[b] += bias_row
    return y
